# revision 2
# baseline (speedup 1.0000x reference)
"""Trainium2 Bass kernel for nn_BoneLinear: out = x @ W^T + pooled(x) @ disha.

Identity used: pooled(x) @ disha == x @ A where A[j, o] = disha[j % 64, o]
(vertical tiling of disha).  So the whole module is one dense matmul:
    out = x @ (W^T + tile(disha, 16))

v2: the PE does ONLY the 16 accumulating fp16 matmuls per token tile
(512-cycle moving streams at 2.4 GHz warm clock = 3.41 us/tile floor).
Everything else is offloaded:
  - x load: SWDGE (gpsimd) DMA with in-flight f32->fp16 cast (HBM read
    f32, SBUF write fp16) -- no ACT/DVE cast pass, half the SBUF write
    traffic of a f32 load.
  - x transpose: DMA xbar transpose (dma_start_transpose via
    nc.sync.dma_start(transpose=True)), one instruction per token tile,
    SBUF->SBUF fp16, [128,1024] -> [128,8,128] with
    xT[p, kc, t] = x[t, kc*128+p].  ~0.9 us/tile on the DMA engines,
    replacing ~1.8 us/tile of PE transpose work in v1.
  - PSUM->SBUF out copies on ACT/DVE (scheduler-balanced), store on the
    ACT HWDGE ring (transposes ride the SP ring) so the two DMA FIFOs
    don't serialize each other.
Per-tile budgets: PE 3.41 us; HBM 1.0 MB = 2.9 us; SBUF DMA ports
1.25 MB = 2.9 us; Pool/Q7 SWDGE emission ~0.5 us (pair loads).
  fp16 operands measured relmax ~3.3e-4 vs the fp32 reference.

Setup (one-time, outside the timed loop): load W naturally, PE-transpose
it (4 transposes packed per PSUM bank), add the partition-tiled disha,
round to fp16 -> W_eff^T resident in SBUF [128, 8, 1024].

Sharding: pure data-parallel over batch (B=8 -> one batch element per
core).  Each core reads its x shard [4096, 1024], full weight and disha,
and writes its output shard [4096, 1024].  No collectives.
"""

import sys
import os

for _p in ("/opt/trn_rl_repo", "/root/.axon_site/_ro/trn_rl_repo"):
    if os.path.isdir(_p) and _p not in sys.path:
        sys.path.insert(0, _p)

import numpy as np

import concourse.bass as bass
import concourse.mybir as mybir
import concourse.tile as tile
from concourse import bacc
from concourse.bass_utils import run_bass_kernel_spmd
from concourse.masks import make_identity

# Problem shapes (hardcoded per contract)
B, S, D_IN, D_OUT, R = 8, 4096, 1024, 1024, 64
N_CORES = 8
P = 128
KO = D_IN // P          # 8 contraction chunks of 128
OC = D_OUT // P         # 8 output chunks of 128 (for W transpose)
MT = S // P             # 32 token tiles per core
NF = 512                # matmul moving free dim (one PSUM bank of fp32)
NT = D_OUT // NF        # 2 n-tiles

F32 = mybir.dt.float32
F16 = mybir.dt.float16
MM_DT = F16


def build_bass(reps: int = 1, loop: int = 1, pw: int = 2,
               use_dma_cast: bool = True, use_xbar_t: bool = True,
               xh_bufs: int = 3, xt_bufs: int = 4, o_bufs: int = 4,
               acc_bufs: int = 3):
    """reps>1 (python-unrolled) or loop>1 (hardware For_i) repeat the
    steady-state compute inside the NEFF; used only for wall-clock
    differencing in benchmarks (the graded kernel uses reps=1, loop=1)."""
    nc = bacc.Bacc("TRN2", target_bir_lowering=False, debug=False, num_devices=1)
    x_ap = nc.dram_tensor("x", [S, D_IN], F32, kind="ExternalInput").ap()
    w_ap = nc.dram_tensor("w", [D_OUT, D_IN], F32, kind="ExternalInput").ap()
    d_ap = nc.dram_tensor("disha", [R, D_OUT], F32, kind="ExternalInput").ap()
    out_ap = nc.dram_tensor("out", [S, D_OUT], F32, kind="ExternalOutput").ap()

    with tile.TileContext(nc) as tc:
        with (
            tc.tile_pool(name="const", bufs=1) as const,
            tc.tile_pool(name="wp", bufs=1) as wpool,
            tc.tile_pool(name="xh", bufs=xh_bufs) as xhpool,
            tc.tile_pool(name="xtp", bufs=xt_bufs) as xtpool,
            tc.tile_pool(name="op", bufs=o_bufs) as opool,
        ):
            ident = const.tile([P, P], MM_DT)
            make_identity(nc, ident)

            # disha tiled twice on partitions: disha2[p, :] = disha[p % 64, :]
            disha2f = const.tile([P, D_OUT], F32)
            nc.sync.dma_start(disha2f[0:R, :], d_ap[:, :])
            nc.sync.dma_start(disha2f[R : 2 * R, :], d_ap[:, :])
            disha2 = const.tile([P, D_OUT], MM_DT)
            nc.any.tensor_copy(disha2[:], disha2f[:])

            # Build W_eff^T[p + 128*kc, oc*128 + q] = W[q(of oc), p(of kc)] + disha2[p]
            # 4 PE transposes packed per PSUM bank (one accumulation group),
            # then a single wide DVE add per bank.  One-time setup.
            GRP = NF // P  # 4 transposes per bank
            w_eff = wpool.tile([P, KO, D_OUT], MM_DT)
            with (
                tc.tile_pool(name="wnat", bufs=1) as wnat_pool,
                tc.tile_pool(name="pstp", bufs=4, space="PSUM") as psum_tp,
            ):
                w_nat = wnat_pool.tile([P, OC, D_IN], F32)
                w_nath = wnat_pool.tile([P, OC, D_IN], MM_DT)
                w_src = w_ap.rearrange("(oc p) d -> p oc d", p=P)
                for kc in range(KO):
                    nc.sync.dma_start(
                        w_nat[:, :, kc * P : (kc + 1) * P],
                        w_src[:, :, kc * P : (kc + 1) * P],
                    )
                    nc.any.tensor_copy(
                        w_nath[:, :, kc * P : (kc + 1) * P],
                        w_nat[:, :, kc * P : (kc + 1) * P],
                    )
                for kc in range(KO):
                    for og in range(OC // GRP):
                        pst = psum_tp.tile([P, NF], MM_DT, tag="tp")
                        for j in range(GRP):
                            oc = og * GRP + j
                            nc.tensor.matmul(
                                pst[:, j * P : (j + 1) * P],
                                w_nath[:, oc, kc * P : (kc + 1) * P],
                                ident[:],
                                is_transpose=True,
                                start=(j == 0),
                                stop=(j == GRP - 1),
                            )
                        nc.vector.tensor_add(
                            w_eff[:, kc, og * NF : (og + 1) * NF],
                            pst[:],
                            disha2[:, og * NF : (og + 1) * NF],
                        )

            # Main loop over token tiles
            import contextlib

            with tc.tile_pool(name="psacc", bufs=acc_bufs, space="PSUM") as psum_acc:
                loop_cm = (
                    tc.For_i(0, loop, 1) if loop > 1 else contextlib.nullcontext()
                )
                with loop_cm:
                    for rep in range(reps):

                        def emit_load(mp, rep=rep):
                            """SWDGE-load pw token tiles, casting f32->fp16
                            in the DMA datapath."""
                            x_h = xhpool.tile(
                                [P, pw, D_IN], MM_DT, tag="x_h",
                                name=f"x_{rep}_{mp}",
                            )
                            src = x_ap[
                                mp * pw * P : (mp + 1) * pw * P, :
                            ].rearrange("(two p) d -> p two d", two=pw)
                            if use_dma_cast:
                                nc.gpsimd.dma_start(x_h[:], src)
                            else:
                                x_t = xhpool.tile(
                                    [P, pw, D_IN], F32, tag="x_t",
                                    name=f"xf_{rep}_{mp}",
                                )
                                nc.sync.dma_start(x_t[:], src)
                                nc.any.tensor_copy(x_h[:], x_t[:])
                            return x_h

                        def emit_transpose(x_h, t, m, rep=rep):
                            """xbar-transpose token tile m (= slot t of its
                            load group): xT[p, kc, tok] = x[tok, kc*128+p]."""
                            xT = xtpool.tile(
                                [P, KO, P], MM_DT, tag="xT",
                                name=f"xT_{rep}_{m}",
                            )
                            nc.sync.dma_start(
                                xT[:], x_h[:, t, :], transpose=True
                            )
                            return xT

                        xh_groups = {0: emit_load(0)}
                        xT_cur = emit_transpose(xh_groups[0], 0, 0)
                        for m in range(MT):
                            if (m + 1) % pw == 0 and (m + 1) // pw < MT // pw:
                                xh_groups[(m + 1) // pw] = emit_load(
                                    (m + 1) // pw
                                )
                            xT_next = (
                                emit_transpose(
                                    xh_groups[(m + 1) // pw],
                                    (m + 1) % pw,
                                    m + 1,
                                )
                                if m + 1 < MT
                                else None
                            )

                            pss = [
                                psum_acc.tile(
                                    [P, NF], F32, tag=f"acc{n}",
                                    name=f"acc_{rep}_{m}_{n}",
                                )
                                for n in range(NT)
                            ]
                            for kc in range(KO):
                                for n in range(NT):
                                    nc.tensor.matmul(
                                        pss[n][:],
                                        xT_cur[:, kc],
                                        w_eff[:, kc, n * NF : (n + 1) * NF],
                                        start=(kc == 0),
                                        stop=(kc == KO - 1),
                                    )
                            o_sb = opool.tile(
                                [P, D_OUT], F32, tag="o", name=f"o_{rep}_{m}"
                            )
                            for n in range(NT):
                                nc.any.tensor_copy(
                                    o_sb[:, n * NF : (n + 1) * NF], pss[n][:]
                                )
                            # store on the ACT HWDGE ring; transposes ride SP
                            nc.scalar.dma_start(
                                out_ap[m * P : (m + 1) * P, :], o_sb[:]
                            )
                            xT_cur = xT_next

    nc.compile()
    return nc


def kernel(x: np.ndarray, weight: np.ndarray, disha: np.ndarray) -> np.ndarray:
    assert x.shape == (B, S, D_IN) and weight.shape == (D_OUT, D_IN)
    assert disha.shape == (R, D_OUT)
    x = np.ascontiguousarray(x, dtype=np.float32)
    weight = np.ascontiguousarray(weight, dtype=np.float32)
    disha = np.ascontiguousarray(disha, dtype=np.float32)
    in_maps = [
        {"x": x[c], "w": weight, "disha": disha} for c in range(N_CORES)
    ]
    # The axon-proxied exec occasionally dies with NRT_EXEC_UNIT_UNRECOVERABLE
    # on an otherwise-good NEFF; retry a couple of times with a fresh build.
    last_exc = None
    for attempt in range(3):
        try:
            nc = build_bass()
            res = run_bass_kernel_spmd(
                nc, in_maps, core_ids=list(range(N_CORES))
            )
            break
        except Exception as e:  # noqa: BLE001
            last_exc = e
            import time as _time

            _time.sleep(5.0 * (attempt + 1))
    else:
        raise last_exc
    out = np.stack([res.results[c]["out"] for c in range(N_CORES)], axis=0)
    return out


if __name__ == "__main__":
    rng = np.random.default_rng(0)
    x = rng.standard_normal((B, S, D_IN), dtype=np.float32)
    w = (rng.standard_normal((D_OUT, D_IN), dtype=np.float32) / 32.0).astype(
        np.float32
    )
    d = (rng.standard_normal((R, D_OUT), dtype=np.float32) * 0.01).astype(np.float32)
    out = kernel(x=x, weight=w, disha=d)
    print(out.shape, out.dtype)
